# revision 44
# baseline (speedup 1.0000x reference)
"""Trainium2 Bass kernel: batched 8x8-block IDCT (dequant + 2D separable transform).

Math per 8x8 block b of each 1024x1024 image:
    out_b = mtx.T @ (qtable * b) @ mtx + 128

Single-pass vec-64 formulation: flatten each 8x8 block to a 64-vector
(row-major, p = 8i+j). Then

    vec(out_b) = [diag(vec(qtable)) @ (mtx (x) mtx)]^T @ vec(b) + 128

i.e. one 64x64 matrix Kq applied to every block, with the dequant folded
into the matrix. Two independent copies of Kq stacked block-diagonally
use the full 128x128 PE array, so one fp16 matmul pass with a stationary
weight loaded once processes two blocks per moving column:

  - Host packs x into a [128, 32768] fp16 tensor per core: partition
    p = 64t + 8i + j, free = (img, blockrow, blockcol//2), t = blockcol%2.
  - Device: DMA chunk in -> matmul (K2 stationary, data moving, PSUM fp32)
    -> drain PSUM to SBUF as quantized uint8 (alternating ACT/DVE) -> DMA out.
  - Host rescales the uint8 output and unpacks back to image layout.

fp16 moving data: 1 PE cycle/row (vs 4 for fp32) and half the HBM read
traffic; uint8 output quarters the HBM write traffic. fp32 accumulation
in PSUM. The uint8 affine scale/bias ride in as a tiny [128,2] input so
the compiled NEFF is input-independent.
"""

import numpy as np

_N_CORES = 8
_B, _H, _W = 32, 1024, 1024
_PER = _B // _N_CORES                  # images per core
_COLS = _PER * (_H // 8) * (_W // 16)  # 32768 free columns per core
_PCHUNK = 2048                         # columns per PSUM tile / drain (one 8KB PSUM slot)
_MMW = 512                             # moving free width per matmul
# Input DMA chunk sizes: big chunks for the body (amortize DGE overhead),
# tapered tail so the final load->matmul->drain->store chain is short.
_CHUNKS = [1024, 2048, 4096, 8192, 8192, 4096, 2048, 1024, 1024, 512, 512]
assert sum(_CHUNKS) == _COLS

# Host-side reconstruction offset in LSBs: 0.5 if the device fp32->uint8
# conversion truncates, 0.0 if it rounds to nearest.
_C_OFF = 0.0

# On-device dtype for the packed input and the folded weights.
_USE_BF16 = False


def _np_in_dtype():
    if _USE_BF16:
        import ml_dtypes

        return ml_dtypes.bfloat16
    return np.float16

_nc_cache = None


def _build_nc():
    from contextlib import ExitStack

    import concourse.bass as bass
    import concourse.tile as tile
    from concourse import mybir

    F16 = mybir.dt.bfloat16 if _USE_BF16 else mybir.dt.float16
    F32 = mybir.dt.float32
    U8 = mybir.dt.uint8
    nc = bass.Bass()
    x_in = nc.declare_dram_parameter("xv", [128, _COLS], F16, isOutput=False)
    k_in = nc.declare_dram_parameter("k2", [128, 128], F16, isOutput=False)
    sb_in = nc.declare_dram_parameter("sb", [128, 2], F32, isOutput=False)
    y_out = nc.declare_dram_parameter("y", [128, _COLS], U8, isOutput=True)

    with ExitStack() as ctx:
        tc = ctx.enter_context(tile.TileContext(nc))
        const = ctx.enter_context(tc.tile_pool(name="const", bufs=1))
        xp = ctx.enter_context(tc.tile_pool(name="xp", bufs=5))
        op = ctx.enter_context(tc.tile_pool(name="op", bufs=4))
        pp = ctx.enter_context(tc.tile_pool(name="pp", bufs=2, space="PSUM"))

        # The first data chunk goes to the head of the SP queue so the
        # first matmul isn't stuck behind the constant loads.
        xt0 = xp.tile([128, max(_CHUNKS)], F16, tag="xt")
        nc.sync.dma_start(xt0[:, : _CHUNKS[0]], x_in[:, : _CHUNKS[0]])

        k2 = const.tile([128, 128], F16)
        nc.sync.dma_start(k2[:], k_in[:])
        sb = const.tile([128, 2], F32)
        nc.sync.dma_start(sb[:], sb_in[:])
        scale = sb[:, 0:1]
        bias = sb[:, 1:2]

        # Load the stationary weights into the PE array exactly once (this
        # also absorbs k2's DMA wait); every matmul below is marked
        # non-self-loading (ldweights=False) and reuses the resident array.
        nc.tensor.ldweights(k2[:])

        drain = 0
        c0 = 0
        for chunk in _CHUNKS:
            if c0 == 0:
                xt = xt0
            else:
                xt = xp.tile([128, max(_CHUNKS)], F16, tag="xt")
                nc.sync.dma_start(xt[:, :chunk], x_in[:, c0 : c0 + chunk])

            for q0 in range(0, chunk, _PCHUNK):
                pc = min(_PCHUNK, chunk - q0)
                pt = pp.tile([128, _PCHUNK], F32, tag="pt")
                for m0 in range(0, pc, _MMW):
                    mw = min(_MMW, pc - m0)
                    mm = nc.tensor.matmul(
                        pt[:, m0 : m0 + mw],
                        k2[:],
                        xt[:, q0 + m0 : q0 + m0 + mw],
                        start=True,
                        stop=True,
                    )
                    mm.ins.ldweights = False
                ot = op.tile([128, _PCHUNK], U8, tag="ot")
                if drain % 2 == 0:
                    nc.scalar.activation(
                        ot[:, :pc],
                        pt[:, :pc],
                        mybir.ActivationFunctionType.Identity,
                        bias=bias,
                        scale=scale,
                    )
                else:
                    nc.vector.tensor_scalar(
                        ot[:, :pc], pt[:, :pc], scale, bias,
                        mybir.AluOpType.mult, mybir.AluOpType.add,
                    )
                drain += 1
                nc.gpsimd.dma_start(y_out[:, c0 + q0 : c0 + q0 + pc], ot[:, :pc])
            c0 += chunk

    _split_excess_waits(nc, mybir)
    return nc


def _split_excess_waits(nc, mybir):
    """Walrus allows a limited number of sync waits per lowered instruction
    (1 for DMA/DVE/ACT structs, a couple for matmul via the LDWEIGHTS pair,
    2 per EventSemaphore). Tile's wait assignment can attach more; move the
    excess onto standalone same-engine EventSemaphore carriers."""

    def budget(inst):
        tn = type(inst).__name__
        if tn == "InstEventSemaphore":
            return 2
        return 1

    wid = 0
    for fn in nc.m.functions:
        for bb in fn.blocks:
            out = []
            for inst in bb.instructions:
                si = inst.sync_info
                waits = list(si.on_wait) if si is not None else []
                b = budget(inst)
                if len(waits) > b:
                    extra, keep = waits[:-b], waits[-b:]
                    for i in range(0, len(extra), 2):
                        ev = mybir.InstEventSemaphore(
                            name=f"WSPLIT-{wid}", ins=[], outs=[]
                        )
                        wid += 1
                        ev.engine = inst.engine
                        ev.sync_info = mybir.SyncInfo(
                            on_wait=extra[i : i + 2], on_update=[]
                        )
                        out.append(ev)
                    inst.sync_info = mybir.SyncInfo(
                        on_wait=keep, on_update=list(si.on_update)
                    )
                out.append(inst)
            bb.instructions = out


def _get_nc():
    global _nc_cache
    if _nc_cache is None:
        _nc_cache = _build_nc()
    return _nc_cache


def _pack_inputs(x, qtable, mtx):
    # x image layout -> per-core [128, _COLS] fp16 block-vector layout.
    # row = 8r+i, col = 16c2+8t+j; partition = 64t+8i+j, free = (b4, r, c2).
    idt = _np_in_dtype()
    xh = np.asarray(x, dtype=np.float32).reshape(_N_CORES, _PER, 128, 8, 64, 2, 8)
    xv = np.ascontiguousarray(
        xh.astype(idt).transpose(0, 5, 3, 6, 1, 2, 4).reshape(_N_CORES, 128, _COLS)
    )
    # K2 = blkdiag(Kq, Kq), Kq = diag(vec(qtable)) @ kron(mtx, mtx)
    q64 = np.asarray(qtable, dtype=np.float32).reshape(64)
    kq = (q64[:, None] * np.kron(mtx, mtx)).astype(np.float32)
    k2 = np.zeros((128, 128), np.float32)
    k2[:64, :64] = kq
    k2[64:, 64:] = kq

    # Certified bound on |out - 128| = |Kq^T xblk|: max block L2 norm times
    # max column L2 norm of Kq (2% headroom for fp16 input rounding).
    x32 = xv.astype(np.float32)
    bn = np.sqrt(
        np.maximum(
            (x32[:, :64, :] ** 2).sum(axis=1).max(),
            (x32[:, 64:, :] ** 2).sum(axis=1).max(),
        )
    )
    kc = np.sqrt((kq.astype(np.float64) ** 2).sum(axis=0)).max()
    amp = 1.02 * float(bn) * float(kc) + 1.0
    s = 2.0 * amp / 255.0
    # device: u8 = convert(psum * (1/s) + (amp/s));  host: out = 128 + (u8 - amp/s + c)*s
    return xv, k2.astype(idt), np.float32(1.0 / s), np.float32(amp / s), s, amp


def _unpack_output(y, s, amp):
    # y: [_N_CORES, 128, _COLS] uint8 -> full image layout fp32
    out = (
        y.reshape(_N_CORES, 2, 8, 8, _PER, 128, 64)
        .transpose(0, 4, 5, 2, 6, 1, 3)
        .reshape(_B, 1, _H, _W)
        .astype(np.float32)
    )
    return out * np.float32(s) + np.float32(128.0 - amp + _C_OFF * s)


def _run(x, qtable, mtx, trace=False, **kwargs):
    from concourse.bass_utils import run_bass_kernel_spmd

    qtable = np.asarray(qtable, dtype=np.float32)
    mtx = np.asarray(mtx, dtype=np.float32)
    xv, k2, dev_scale, dev_bias, s, amp = _pack_inputs(x, qtable, mtx)
    sb = np.ascontiguousarray(
        np.broadcast_to(np.array([dev_scale, dev_bias], np.float32), (128, 2))
    )

    in_maps = [{"xv": xv[i], "k2": k2, "sb": sb} for i in range(_N_CORES)]
    res = run_bass_kernel_spmd(
        _get_nc(), in_maps, list(range(_N_CORES)), trace=trace, **kwargs
    )
    y = np.stack([res.results[i]["y"] for i in range(_N_CORES)], axis=0)
    return _unpack_output(y, s, amp), res


def kernel(x, qtable, mtx):
    out, _ = _run(x, qtable, mtx, trace=False)
    return out


# revision 45
# speedup vs baseline: 1.0256x; 1.0256x over previous
"""Trainium2 Bass kernel: batched 8x8-block IDCT (dequant + 2D separable transform).

Math per 8x8 block b of each 1024x1024 image:
    out_b = mtx.T @ (qtable * b) @ mtx + 128

Single-pass vec-64 formulation: flatten each 8x8 block to a 64-vector
(row-major, p = 8i+j). Then

    vec(out_b) = [diag(vec(qtable)) @ (mtx (x) mtx)]^T @ vec(b) + 128

i.e. one 64x64 matrix Kq applied to every block, with the dequant folded
into the matrix. Two independent copies of Kq stacked block-diagonally
use the full 128x128 PE array, so one fp16 matmul pass with a stationary
weight loaded once processes two blocks per moving column:

  - Host packs x into a [128, 32768] fp16 tensor per core: partition
    p = 64t + 8i + j, free = (img, blockrow, blockcol//2), t = blockcol%2.
  - Device: DMA chunk in -> matmul (K2 stationary, data moving, PSUM fp32)
    -> drain PSUM to SBUF as quantized uint8 (alternating ACT/DVE) -> DMA out.
  - Host rescales the uint8 output and unpacks back to image layout.

fp16 moving data: 1 PE cycle/row (vs 4 for fp32) and half the HBM read
traffic; uint8 output quarters the HBM write traffic. fp32 accumulation
in PSUM. The uint8 affine scale/bias ride in as a tiny [128,2] input so
the compiled NEFF is input-independent.
"""

import numpy as np

_N_CORES = 8
_B, _H, _W = 32, 1024, 1024
_PER = _B // _N_CORES                  # images per core
_COLS = _PER * (_H // 8) * (_W // 16)  # 32768 free columns per core
_PCHUNK = 2048                         # columns per PSUM tile / drain (one 8KB PSUM slot)
_MMW = 512                             # moving free width per matmul
# Input DMA chunk sizes: big chunks for the body (amortize DGE overhead),
# tapered tail so the final load->matmul->drain->store chain is short.
_CHUNKS = [1024, 2048, 4096, 8192, 8192, 4096, 2048, 1024, 1024, 512, 512]
assert sum(_CHUNKS) == _COLS

# Host-side reconstruction offset in LSBs: 0.5 if the device fp32->uint8
# conversion truncates, 0.0 if it rounds to nearest.
_C_OFF = 0.0

# On-device dtype for the packed input and the folded weights.
_USE_BF16 = False


def _np_in_dtype():
    if _USE_BF16:
        import ml_dtypes

        return ml_dtypes.bfloat16
    return np.float16

_nc_cache = None


def _build_nc():
    from contextlib import ExitStack

    import concourse.bass as bass
    import concourse.tile as tile
    from concourse import mybir

    F16 = mybir.dt.bfloat16 if _USE_BF16 else mybir.dt.float16
    F32 = mybir.dt.float32
    U8 = mybir.dt.uint8
    nc = bass.Bass()
    x_in = nc.declare_dram_parameter("xv", [128, _COLS], F16, isOutput=False)
    k_in = nc.declare_dram_parameter("k2", [128, 128], F16, isOutput=False)
    sb_in = nc.declare_dram_parameter("sb", [128, 2], F32, isOutput=False)
    y_out = nc.declare_dram_parameter("y", [128, _COLS], U8, isOutput=True)

    with ExitStack() as ctx:
        tc = ctx.enter_context(tile.TileContext(nc))
        const = ctx.enter_context(tc.tile_pool(name="const", bufs=1))
        xp = ctx.enter_context(tc.tile_pool(name="xp", bufs=4))
        op = ctx.enter_context(tc.tile_pool(name="op", bufs=4))
        pp = ctx.enter_context(tc.tile_pool(name="pp", bufs=2, space="PSUM"))

        # The first data chunk goes to the head of the SP queue so the
        # first matmul isn't stuck behind the constant loads.
        xt0 = xp.tile([128, max(_CHUNKS)], F16, tag="xt")
        nc.sync.dma_start(xt0[:, : _CHUNKS[0]], x_in[:, : _CHUNKS[0]])

        k2 = const.tile([128, 128], F16)
        nc.sync.dma_start(k2[:], k_in[:])
        sb = const.tile([128, 2], F32)
        nc.sync.dma_start(sb[:], sb_in[:])
        scale = sb[:, 0:1]
        bias = sb[:, 1:2]

        # Load the stationary weights into the PE array exactly once (this
        # also absorbs k2's DMA wait); every matmul below is marked
        # non-self-loading (ldweights=False) and reuses the resident array.
        nc.tensor.ldweights(k2[:])

        drain = 0
        c0 = 0
        for chunk in _CHUNKS:
            if c0 == 0:
                xt = xt0
            else:
                xt = xp.tile([128, max(_CHUNKS)], F16, tag="xt")
                nc.sync.dma_start(xt[:, :chunk], x_in[:, c0 : c0 + chunk])

            for q0 in range(0, chunk, _PCHUNK):
                pc = min(_PCHUNK, chunk - q0)
                pt = pp.tile([128, _PCHUNK], F32, tag="pt")
                for m0 in range(0, pc, _MMW):
                    mw = min(_MMW, pc - m0)
                    mm = nc.tensor.matmul(
                        pt[:, m0 : m0 + mw],
                        k2[:],
                        xt[:, q0 + m0 : q0 + m0 + mw],
                        start=True,
                        stop=True,
                    )
                    mm.ins.ldweights = False
                ot = op.tile([128, _PCHUNK], U8, tag="ot")
                if drain % 2 == 0:
                    nc.scalar.activation(
                        ot[:, :pc],
                        pt[:, :pc],
                        mybir.ActivationFunctionType.Identity,
                        bias=bias,
                        scale=scale,
                    )
                else:
                    nc.vector.tensor_scalar(
                        ot[:, :pc], pt[:, :pc], scale, bias,
                        mybir.AluOpType.mult, mybir.AluOpType.add,
                    )
                drain += 1
                nc.gpsimd.dma_start(y_out[:, c0 + q0 : c0 + q0 + pc], ot[:, :pc])
            c0 += chunk

    _split_excess_waits(nc, mybir)
    return nc


def _split_excess_waits(nc, mybir):
    """Walrus allows a limited number of sync waits per lowered instruction
    (1 for DMA/DVE/ACT structs, a couple for matmul via the LDWEIGHTS pair,
    2 per EventSemaphore). Tile's wait assignment can attach more; move the
    excess onto standalone same-engine EventSemaphore carriers."""

    def budget(inst):
        tn = type(inst).__name__
        if tn == "InstEventSemaphore":
            return 2
        return 1

    wid = 0
    for fn in nc.m.functions:
        for bb in fn.blocks:
            out = []
            for inst in bb.instructions:
                si = inst.sync_info
                waits = list(si.on_wait) if si is not None else []
                b = budget(inst)
                if len(waits) > b:
                    extra, keep = waits[:-b], waits[-b:]
                    for i in range(0, len(extra), 2):
                        ev = mybir.InstEventSemaphore(
                            name=f"WSPLIT-{wid}", ins=[], outs=[]
                        )
                        wid += 1
                        ev.engine = inst.engine
                        ev.sync_info = mybir.SyncInfo(
                            on_wait=extra[i : i + 2], on_update=[]
                        )
                        out.append(ev)
                    inst.sync_info = mybir.SyncInfo(
                        on_wait=keep, on_update=list(si.on_update)
                    )
                out.append(inst)
            bb.instructions = out


def _get_nc():
    global _nc_cache
    if _nc_cache is None:
        _nc_cache = _build_nc()
    return _nc_cache


def _pack_inputs(x, qtable, mtx):
    # x image layout -> per-core [128, _COLS] fp16 block-vector layout.
    # row = 8r+i, col = 16c2+8t+j; partition = 64t+8i+j, free = (b4, r, c2).
    idt = _np_in_dtype()
    xh = np.asarray(x, dtype=np.float32).reshape(_N_CORES, _PER, 128, 8, 64, 2, 8)
    xv = np.ascontiguousarray(
        xh.astype(idt).transpose(0, 5, 3, 6, 1, 2, 4).reshape(_N_CORES, 128, _COLS)
    )
    # K2 = blkdiag(Kq, Kq), Kq = diag(vec(qtable)) @ kron(mtx, mtx)
    q64 = np.asarray(qtable, dtype=np.float32).reshape(64)
    kq = (q64[:, None] * np.kron(mtx, mtx)).astype(np.float32)
    k2 = np.zeros((128, 128), np.float32)
    k2[:64, :64] = kq
    k2[64:, 64:] = kq

    # Certified bound on |out - 128| = |Kq^T xblk|: max block L2 norm times
    # max column L2 norm of Kq (2% headroom for fp16 input rounding).
    x32 = xv.astype(np.float32)
    bn = np.sqrt(
        np.maximum(
            (x32[:, :64, :] ** 2).sum(axis=1).max(),
            (x32[:, 64:, :] ** 2).sum(axis=1).max(),
        )
    )
    kc = np.sqrt((kq.astype(np.float64) ** 2).sum(axis=0)).max()
    amp = 1.02 * float(bn) * float(kc) + 1.0
    s = 2.0 * amp / 255.0
    # device: u8 = convert(psum * (1/s) + (amp/s));  host: out = 128 + (u8 - amp/s + c)*s
    return xv, k2.astype(idt), np.float32(1.0 / s), np.float32(amp / s), s, amp


def _unpack_output(y, s, amp):
    # y: [_N_CORES, 128, _COLS] uint8 -> full image layout fp32
    out = (
        y.reshape(_N_CORES, 2, 8, 8, _PER, 128, 64)
        .transpose(0, 4, 5, 2, 6, 1, 3)
        .reshape(_B, 1, _H, _W)
        .astype(np.float32)
    )
    return out * np.float32(s) + np.float32(128.0 - amp + _C_OFF * s)


def _run(x, qtable, mtx, trace=False, **kwargs):
    from concourse.bass_utils import run_bass_kernel_spmd

    qtable = np.asarray(qtable, dtype=np.float32)
    mtx = np.asarray(mtx, dtype=np.float32)
    xv, k2, dev_scale, dev_bias, s, amp = _pack_inputs(x, qtable, mtx)
    sb = np.ascontiguousarray(
        np.broadcast_to(np.array([dev_scale, dev_bias], np.float32), (128, 2))
    )

    in_maps = [{"xv": xv[i], "k2": k2, "sb": sb} for i in range(_N_CORES)]
    res = run_bass_kernel_spmd(
        _get_nc(), in_maps, list(range(_N_CORES)), trace=trace, **kwargs
    )
    y = np.stack([res.results[i]["y"] for i in range(_N_CORES)], axis=0)
    return _unpack_output(y, s, amp), res


def kernel(x, qtable, mtx):
    out, _ = _run(x, qtable, mtx, trace=False)
    return out
